# revision 21
# baseline (speedup 1.0000x reference)
"""GNN message-passing layer (LplsNorm + residual conv) on 8 Trainium2 cores.

Computation (reference, all f32):
    degree = A.sum(-1); ds = degree**-0.5
    mf  = f + ds[:,None] * (A @ (ds[:,None] * f))      # a_norm = ds A ds
    out = relu(mf @ W + b)
Distribution: A row-sharded over 8 cores ([1024, 8192] each), feature
replicated.

v6 schedule (fp8 DoubleRow + split degree AllGather):
  - Phase 1: stream the A shard once (DMA-bound ~330 GB/s; f loads are
    dependency-pinned behind the A stream so they cannot steal phase-1
    bandwidth). ScalarE casts chunks to fp8-e4m3 while accumulating exact
    f32 row sums; TensorE transposes fp8 tiles; DVE copies them into an
    SBUF-resident transposed-A store (8 MiB fp8, no spill).
  - A mid-kernel 4 KB AllGather costs ~50 us wall (mesh firmware latency),
    so degrees ship in TWO collectives: m-tiles 0-3 at ~58% of the stream
    (lands before the stream ends) and m-tiles 4-7 at the end. The second
    collective's window is bridged by real matmuls on the first half's
    k-chunks, with f streaming/parking (raw fp8, no ds needed) running
    underneath; paced dummy transposes keep the PE's HAM clock warm.
  - The A@X term contributes only ~0.7% of mf (the residual dominates), so
    fp8 keeps total l2 rel err ~2e-3 (verified vs numpy; gate 2e-2).
  - Main matmul: DoubleRow fp8 (2 k-chunks/instruction). M-tiles 0-5
    accumulate in 6 PSUM banks; m-tiles 6-7 + epilogue o-accumulators
    rotate through the same pool as banks free up. psA's 2 banks serve
    phase-1 transposes, then the epilogue's mf transposes.
  - Epilogue: mf = Y * dsown/64 + f_res (DVE, bf16), mf @ W in bf16, bias
    via a K=1 bf16 matmul, ACT relu, store.
"""

import numpy as np

import concourse.bass as bass
import concourse.mybir as mybir
import concourse.tile as tile
from concourse import bacc
from concourse import bass_utils
from concourse.masks import make_identity

N = 8192
D = 512
NCORES = 8
P = 128
R = N // NCORES          # rows per core: 1024
MT = R // P              # m-tiles per core: 8
KC = N // P              # k-chunks: 64
ACH = 2048               # A stream chunk width (f32 -> 1 MiB per DMA)
NACH = N // ACH          # stream chunks per row-block: 4
GPC = ACH // (4 * P)     # transpose groups (of 4 tiles) per stream chunk: 4
MTG = 6                  # m-tiles in the big matmul group (PSUM banks)
HMT = 4                  # m-tiles per degree-collective half

F32 = mybir.dt.float32
BF16 = mybir.dt.bfloat16
F8 = mybir.dt.float8e4

_NC_CACHE = {}


def _build():
    nc = bacc.Bacc("TRN2", target_bir_lowering=False, debug=False, num_devices=NCORES)

    a_d = nc.dram_tensor("a", [R, N], F32, kind="ExternalInput")
    f_d = nc.dram_tensor("f", [N, D], F32, kind="ExternalInput")
    fres_d = nc.dram_tensor("fres", [R, D], F32, kind="ExternalInput")
    w_d = nc.dram_tensor("w", [D, D], F32, kind="ExternalInput")
    b_d = nc.dram_tensor("bias", [1, D], F32, kind="ExternalInput")
    out_d = nc.dram_tensor("out", [R, D], F32, kind="ExternalOutput")

    AX = mybir.AxisListType.X
    ALU = mybir.AluOpType
    ACT = mybir.ActivationFunctionType
    DR = mybir.MatmulPerfMode.DoubleRow

    with tile.TileContext(nc) as tc:
        with (
            tc.tile_pool(name="const", bufs=1) as constp,
            tc.tile_pool(name="deg", bufs=1) as degp,
            tc.tile_pool(name="astream", bufs=3) as astreamp,
            tc.tile_pool(name="small", bufs=2) as smallp,
            tc.tile_pool(name="atres", bufs=1) as atresp,
            tc.tile_pool(name="xp", bufs=1) as xpp,
            tc.tile_pool(name="f8p", bufs=1) as f8pool,
            tc.tile_pool(name="fstream", bufs=4) as fstreamp,
            tc.tile_pool(name="epi", bufs=2) as epip,
            tc.tile_pool(name="mfp", bufs=MT) as mfpool,
            tc.tile_pool(name="mft", bufs=2) as mftp,
            tc.tile_pool(name="psA", bufs=2, space="PSUM") as psA,      # transposes
            tc.tile_pool(name="psY", bufs=MTG, space="PSUM") as psY,    # Y + o accum
            tc.tile_pool(name="dram", bufs=1, space="DRAM") as dramp,
        ):
            # ---- constants ----
            identity = constp.tile([P, P], F32)
            make_identity(nc, identity[:])
            identity_f8 = constp.tile([P, P], F8)
            make_identity(nc, identity_f8[:])
            identity_bf = constp.tile([P, P], BF16)
            make_identity(nc, identity_bf[:])
            ones_row = constp.tile([1, P], BF16)
            nc.gpsimd.memset(ones_row[:], 1.0)
            bf_sb = constp.tile([1, D], F32)
            nc.sync.dma_start(bf_sb[:], b_d.ap())
            b_sb = constp.tile([1, D], BF16)
            nc.vector.tensor_copy(b_sb[:], bf_sb[:])
            w_sb = constp.tile([P, 4 * D], BF16)  # w chunk wc at [:, wc*D:(wc+1)*D]
            for wc in range(4):
                wf_ch = smallp.tile([P, D], F32, tag="wf")
                nc.sync.dma_start(wf_ch[:], w_d.ap()[wc * P : (wc + 1) * P, :])
                nc.vector.tensor_copy(w_sb[:, wc * D : (wc + 1) * D], wf_ch[:])

            # resident transposed-A store: (mt, kc) tile at col (mt*KC+kc)*P
            at_res = atresp.tile([P, MT * KC * P], F8)
            cin = [dramp.tile([P, HMT], F32, name=f"cin{h}") for h in range(2)]
            cout = [
                dramp.tile([NCORES * P, HMT], F32, name=f"cout{h}") for h in range(2)
            ]

            degree_sb = degp.tile([P, MT], F32)  # col mt = degree of rows mt*128..
            # ds64_sb[p, e*8 + mt] = 64 / sqrt(degree[global row e*1024+mt*128+p])
            ds64_sb = degp.tile([P, KC], F32)
            ds64_v = ds64_sb[:].rearrange("p (e c) -> p e c", e=NCORES)

            def issue_degree_half(h):
                """AllGather degree cols [h*4, h*4+4) directly (no transpose)."""
                nc.sync.dma_start(
                    cin[h][:], degree_sb[:, h * HMT : (h + 1) * HMT]
                )
                nc.gpsimd.collective_compute(
                    "AllGather",
                    ALU.bypass,
                    ins=[cin[h].opt()],
                    outs=[cout[h].opt()],
                    replica_groups=[list(range(NCORES))],
                )

            def consume_degree_half(h):
                """cout[h] row e*P+p col t = degree of global row e*1024+(h*4+t)*128+p."""
                degall = smallp.tile(
                    [P, NCORES * HMT], F32, tag="degall", name=f"dga{h}"
                )
                nc.sync.dma_start(
                    degall[:].rearrange("p (e c) -> p e c", e=NCORES),
                    cout[h].rearrange("(e p) t -> p e t", p=P),
                )
                reciph = degp.tile([P, NCORES * HMT], F32, name=f"reciph{h}")
                nc.vector.reciprocal(reciph[:], degall[:])
                nc.scalar.activation(
                    ds64_v[:, :, h * HMT : (h + 1) * HMT],
                    reciph[:].rearrange("p (e c) -> p e c", e=NCORES),
                    ACT.Sqrt,
                    scale=4096.0,
                )

            # ---- merged pass: degree + transpose-all; half-collectives ----
            for mt in range(MT):
                dcols = smallp.tile([P, NACH], F32, tag="dcols")
                for c in range(NACH):
                    ach = astreamp.tile([P, ACH], F32, tag="ach")
                    nc.sync.dma_start(
                        ach[:], a_d.ap()[mt * P : (mt + 1) * P, c * ACH : (c + 1) * ACH]
                    )
                    ach8 = astreamp.tile([P, ACH], F8, tag="ach8", bufs=2)
                    nc.scalar.activation(
                        ach8[:], ach[:], ACT.Copy, accum_out=dcols[:, c : c + 1]
                    )
                    for g in range(GPC):
                        gk = c * GPC + g  # k-group index 0..15
                        trp = psA.tile([P, 4 * P], F32, tag="trp")
                        for q in range(4):
                            nc.tensor.matmul(
                                trp[:, q * P : (q + 1) * P],
                                ach8[:, (g * 4 + q) * P : (g * 4 + q + 1) * P],
                                identity_f8[:],
                            )
                        dst = at_res[
                            :, (mt * KC + gk * 4) * P : (mt * KC + gk * 4 + 4) * P
                        ]
                        nc.vector.tensor_copy(dst, trp[:])
                nc.vector.reduce_sum(degree_sb[:, mt : mt + 1], dcols[:], axis=AX)
                if mt == HMT - 1:
                    issue_degree_half(0)
            issue_degree_half(1)

            recip8 = degp.tile([P, MT], F32)
            nc.vector.reciprocal(recip8[:], degree_sb[:])
            # dsown64[p, mt] = 1 / (64 * sqrt(degree_own[mt*128 + p]))
            dsown64 = degp.tile([P, MT], F32)
            nc.scalar.activation(dsown64[:], recip8[:], ACT.Sqrt, scale=1.0 / 4096.0)

            # ---- f: stream + park as raw fp8 + scale to X'' + matmul, fully
            # interleaved per 2-chunk batch so every engine queue's order
            # matches data arrival. The first fch buffers take a fake WAW dep
            # on degree_sb so the f stream cannot start before the A stream
            # is done.
            f8raw = f8pool.tile([P, KC * D], F8)
            xp_sb = xpp.tile([P, KC * D], F8)  # chunk kc at [:, kc*D:(kc+1)*D]
            # sub-batch (h, e, s) = k-chunks e*8 + h*4 + s*2 + {0,1} = pair
            # pj = 4e + 2h + s
            f_blk = f_d.ap().rearrange(
                "(e h s c p) d -> h e s p c d", h=2, s=2, c=2, p=P
            )

            def mm_pair(y_ap, mt, pj, start, stop):
                """pj = global k-pair index (k-chunks 2*pj, 2*pj+1)."""
                base = (mt * KC + 2 * pj) * P
                at2 = at_res[:, base : base + 2 * P].rearrange(
                    "p (k m) -> p k m", k=2
                )
                xp2 = xp_sb[:, (2 * pj) * D : (2 * pj + 2) * D].rearrange(
                    "p (k n) -> p k n", k=2
                )
                nc.tensor.matmul(
                    y_ap, at2, xp2, start=start, stop=stop, perf_mode=DR
                )

            ys = [
                psY.tile([P, D], F32, tag="y", name=f"y{i}") for i in range(MTG)
            ]
            # m-tiles 6,7 accumulate in psA's banks (free once the phase-1
            # transposes finish); their half-0 matmuls bridge the second
            # collective's latency window
            ys2 = [
                psA.tile([P, D], F32, tag="trp", name=f"y{MTG + i}")
                for i in range(MT - MTG)
            ]

            def g2_half(h):
                for e in range(NCORES):
                    for s in range(2):
                        pj = 4 * e + 2 * h + s
                        for i, mt in enumerate(range(MTG, MT)):
                            mm_pair(
                                ys2[i][:], mt, pj, pj == 0, pj == KC // 2 - 1
                            )

            nfch = [0]

            def stream_half(h):
                consume_degree_half(h)
                for e in range(NCORES):
                    for s in range(2):
                        pj = 4 * e + 2 * h + s
                        fch = fstreamp.tile([P, 2 * D], F32, tag="fch")
                        if nfch[0] < 4:
                            nc.vector.tensor_copy(
                                fch[:, :1], degree_sb[:, MT - 1 : MT]
                            )
                        nfch[0] += 1
                        nc.sync.dma_start(
                            fch[:].rearrange("p (c d) -> p c d", c=2),
                            f_blk[h, e, s],
                        )
                        for c in range(2):
                            kc = 2 * pj + c
                            # park on DVE (never waits the collective, so the
                            # f stream keeps flowing through its window)
                            nc.vector.tensor_copy(
                                f8raw[:, kc * D : (kc + 1) * D],
                                fch[:, c * D : (c + 1) * D],
                            )
                        for c in range(2):
                            kc = 2 * pj + c
                            # X'' on ScalarE: out = f8 * ds64 (scale is a
                            # per-partition AP)
                            nc.scalar.activation(
                                xp_sb[:, kc * D : (kc + 1) * D],
                                f8raw[:, kc * D : (kc + 1) * D],
                                ACT.Copy,
                                scale=ds64_sb[:, kc : kc + 1],
                            )
                        for mi in range(MTG):
                            mm_pair(ys[mi][:], mi, pj, pj == 0, pj == KC // 2 - 1)

            stream_half(0)
            g2_half(0)
            stream_half(1)
            g2_half(1)

            # epilogue part 1 for group 1: free the Y banks early
            mfs = []
            for mt in range(MTG):
                res = epip.tile([P, D], F32, tag="res")
                nc.sync.dma_start(res[:], fres_d.ap()[mt * P : (mt + 1) * P, :])
                mf = mfpool.tile([P, D], BF16, tag="mf", name=f"mf{mt}")
                nc.vector.scalar_tensor_tensor(
                    mf[:],
                    ys[mt][:],
                    dsown64[:, mt : mt + 1],
                    res[:],
                    op0=ALU.mult,
                    op1=ALU.add,
                )
                mfs.append(mf)
            for i, mt in enumerate(range(MTG, MT)):
                res = epip.tile([P, D], F32, tag="res")
                nc.sync.dma_start(res[:], fres_d.ap()[mt * P : (mt + 1) * P, :])
                mf = mfpool.tile([P, D], BF16, tag="mf", name=f"mf{mt}")
                nc.vector.scalar_tensor_tensor(
                    mf[:],
                    ys2[i][:],
                    dsown64[:, mt : mt + 1],
                    res[:],
                    op0=ALU.mult,
                    op1=ALU.add,
                )
                mfs.append(mf)

            # epilogue part 2: out = relu(mf @ W + b), o accumulators rotate
            # through the freed psY slots
            for mt in range(MT):
                o_ps = psY.tile([P, D], F32, tag="y", name=f"o{mt}")
                for wc in range(4):
                    mfT_ps = psA.tile([P, P], F32, tag="trp")
                    nc.tensor.matmul(
                        mfT_ps[:], mfs[mt][:, wc * P : (wc + 1) * P], identity_bf[:]
                    )
                    mfT_sb = mftp.tile([P, P], BF16, tag="mfT")
                    nc.vector.tensor_copy(mfT_sb[:], mfT_ps[:])
                    nc.tensor.matmul(
                        o_ps[:],
                        mfT_sb[:],
                        w_sb[:, wc * D : (wc + 1) * D],
                        start=(wc == 0),
                        stop=False,
                    )
                nc.tensor.matmul(
                    o_ps[:], ones_row[:], b_sb[:], start=False, stop=True
                )
                osb = epip.tile([P, D], F32, tag="osb")
                nc.scalar.activation(osb[:], o_ps[:], ACT.Relu)
                nc.sync.dma_start(out_d.ap()[mt * P : (mt + 1) * P, :], osb[:])

    nc.compile()
    return nc


def _get_nc():
    if "nc" not in _NC_CACHE:
        _NC_CACHE["nc"] = _build()
    return _NC_CACHE["nc"]


def run(inputs, trace=False, trace_kwargs=None):
    """Run the SPMD kernel; returns (full_output, BassKernelResults)."""
    a = np.ascontiguousarray(np.asarray(inputs["adjacency_matrix"], dtype=np.float32))
    f = np.ascontiguousarray(np.asarray(inputs["feature"], dtype=np.float32))
    w = np.ascontiguousarray(np.asarray(inputs["W"], dtype=np.float32))
    b = np.ascontiguousarray(np.asarray(inputs["b"], dtype=np.float32)).reshape(1, D)

    nc = _get_nc()
    in_maps = []
    for d in range(NCORES):
        rows = slice(d * R, (d + 1) * R)
        in_maps.append({"a": a[rows], "f": f, "fres": f[rows], "w": w, "bias": b})
    res = bass_utils.run_bass_kernel_spmd(
        nc,
        in_maps,
        core_ids=list(range(NCORES)),
        trace=trace,
        **(trace_kwargs or {}),
    )
    out = np.concatenate([r["out"] for r in res.results], axis=0)
    return out, res


def kernel(**inputs):
    out, _ = run(inputs, trace=False)
    return out


# revision 22
# speedup vs baseline: 1.0328x; 1.0328x over previous
"""GNN message-passing layer (LplsNorm + residual conv) on 8 Trainium2 cores.

Computation (reference, all f32):
    degree = A.sum(-1); ds = degree**-0.5
    mf  = f + ds[:,None] * (A @ (ds[:,None] * f))      # a_norm = ds A ds
    out = relu(mf @ W + b)
Distribution: A row-sharded over 8 cores ([1024, 8192] each), feature
replicated.

v6 schedule (fp8 DoubleRow + split degree AllGather):
  - Phase 1: stream the A shard once (DMA-bound ~330 GB/s; f loads are
    dependency-pinned behind the A stream so they cannot steal phase-1
    bandwidth). ScalarE casts chunks to fp8-e4m3 while accumulating exact
    f32 row sums; TensorE transposes fp8 tiles; DVE copies them into an
    SBUF-resident transposed-A store (8 MiB fp8, no spill).
  - A mid-kernel 4 KB AllGather costs ~50 us wall (mesh firmware latency),
    so degrees ship in TWO collectives: m-tiles 0-3 at ~58% of the stream
    (lands before the stream ends) and m-tiles 4-7 at the end. The second
    collective's window is bridged by real matmuls on the first half's
    k-chunks, with f streaming/parking (raw fp8, no ds needed) running
    underneath; paced dummy transposes keep the PE's HAM clock warm.
  - The A@X term contributes only ~0.7% of mf (the residual dominates), so
    fp8 keeps total l2 rel err ~2e-3 (verified vs numpy; gate 2e-2).
  - Main matmul: DoubleRow fp8 (2 k-chunks/instruction). M-tiles 0-5
    accumulate in 6 PSUM banks; m-tiles 6-7 + epilogue o-accumulators
    rotate through the same pool as banks free up. psA's 2 banks serve
    phase-1 transposes, then the epilogue's mf transposes.
  - Epilogue: mf = Y * dsown/64 + f_res (DVE, bf16), mf @ W in bf16, bias
    via a K=1 bf16 matmul, ACT relu, store.
"""

import numpy as np

import concourse.bass as bass
import concourse.mybir as mybir
import concourse.tile as tile
from concourse import bacc
from concourse import bass_utils
from concourse.masks import make_identity

N = 8192
D = 512
NCORES = 8
P = 128
R = N // NCORES          # rows per core: 1024
MT = R // P              # m-tiles per core: 8
KC = N // P              # k-chunks: 64
ACH = 2048               # A stream chunk width (f32 -> 1 MiB per DMA)
NACH = N // ACH          # stream chunks per row-block: 4
GPC = ACH // (4 * P)     # transpose groups (of 4 tiles) per stream chunk: 4
MTG = 6                  # m-tiles in the big matmul group (PSUM banks)
HMT = 4                  # m-tiles per degree-collective half

F32 = mybir.dt.float32
BF16 = mybir.dt.bfloat16
F8 = mybir.dt.float8e4

_NC_CACHE = {}


def _build():
    nc = bacc.Bacc("TRN2", target_bir_lowering=False, debug=False, num_devices=NCORES)

    a_d = nc.dram_tensor("a", [R, N], F32, kind="ExternalInput")
    f_d = nc.dram_tensor("f", [N, D], F32, kind="ExternalInput")
    fres_d = nc.dram_tensor("fres", [R, D], F32, kind="ExternalInput")
    w_d = nc.dram_tensor("w", [D, D], F32, kind="ExternalInput")
    b_d = nc.dram_tensor("bias", [1, D], F32, kind="ExternalInput")
    out_d = nc.dram_tensor("out", [R, D], F32, kind="ExternalOutput")

    AX = mybir.AxisListType.X
    ALU = mybir.AluOpType
    ACT = mybir.ActivationFunctionType
    DR = mybir.MatmulPerfMode.DoubleRow

    with tile.TileContext(nc) as tc:
        with (
            tc.tile_pool(name="const", bufs=1) as constp,
            tc.tile_pool(name="deg", bufs=1) as degp,
            tc.tile_pool(name="astream", bufs=3) as astreamp,
            tc.tile_pool(name="small", bufs=2) as smallp,
            tc.tile_pool(name="atres", bufs=1) as atresp,
            tc.tile_pool(name="xp", bufs=1) as xpp,
            tc.tile_pool(name="f8p", bufs=1) as f8pool,
            tc.tile_pool(name="fstream", bufs=4) as fstreamp,
            tc.tile_pool(name="epi", bufs=2) as epip,
            tc.tile_pool(name="mfp", bufs=MT) as mfpool,
            tc.tile_pool(name="mft", bufs=2) as mftp,
            tc.tile_pool(name="psA", bufs=2, space="PSUM") as psA,      # transposes
            tc.tile_pool(name="psY", bufs=MTG, space="PSUM") as psY,    # Y + o accum
            tc.tile_pool(name="dram", bufs=1, space="DRAM") as dramp,
        ):
            # ---- constants ----
            identity = constp.tile([P, P], F32)
            make_identity(nc, identity[:])
            identity_f8 = constp.tile([P, P], F8)
            make_identity(nc, identity_f8[:])
            identity_bf = constp.tile([P, P], BF16)
            make_identity(nc, identity_bf[:])
            ones_row = constp.tile([1, P], BF16)
            nc.gpsimd.memset(ones_row[:], 1.0)
            bf_sb = constp.tile([1, D], F32)
            nc.sync.dma_start(bf_sb[:], b_d.ap())
            b_sb = constp.tile([1, D], BF16)
            nc.vector.tensor_copy(b_sb[:], bf_sb[:])
            w_sb = constp.tile([P, 4 * D], BF16)  # w chunk wc at [:, wc*D:(wc+1)*D]
            for wc in range(4):
                wf_ch = smallp.tile([P, D], F32, tag="wf")
                nc.sync.dma_start(wf_ch[:], w_d.ap()[wc * P : (wc + 1) * P, :])
                nc.vector.tensor_copy(w_sb[:, wc * D : (wc + 1) * D], wf_ch[:])

            # resident transposed-A store: (mt, kc) tile at col (mt*KC+kc)*P
            at_res = atresp.tile([P, MT * KC * P], F8)
            cin = [dramp.tile([P, HMT], F32, name=f"cin{h}") for h in range(2)]
            cout = [
                dramp.tile([NCORES * P, HMT], F32, name=f"cout{h}") for h in range(2)
            ]

            degree_sb = degp.tile([P, MT], F32)  # col mt = degree of rows mt*128..
            # ds64_sb[p, e*8 + mt] = 64 / sqrt(degree[global row e*1024+mt*128+p])
            ds64_sb = degp.tile([P, KC], F32)
            ds64_v = ds64_sb[:].rearrange("p (e c) -> p e c", e=NCORES)

            def issue_degree_half(h):
                """AllGather degree cols [h*4, h*4+4) directly (no transpose)."""
                nc.sync.dma_start(
                    cin[h][:], degree_sb[:, h * HMT : (h + 1) * HMT]
                )
                nc.gpsimd.collective_compute(
                    "AllGather",
                    ALU.bypass,
                    ins=[cin[h].opt()],
                    outs=[cout[h].opt()],
                    replica_groups=[list(range(NCORES))],
                )

            def consume_degree_half(h):
                """cout[h] row e*P+p col t = degree of global row e*1024+(h*4+t)*128+p."""
                degall = smallp.tile(
                    [P, NCORES * HMT], F32, tag="degall", name=f"dga{h}"
                )
                nc.sync.dma_start(
                    degall[:].rearrange("p (e c) -> p e c", e=NCORES),
                    cout[h].rearrange("(e p) t -> p e t", p=P),
                )
                reciph = degp.tile([P, NCORES * HMT], F32, name=f"reciph{h}")
                nc.vector.reciprocal(reciph[:], degall[:])
                nc.scalar.activation(
                    ds64_v[:, :, h * HMT : (h + 1) * HMT],
                    reciph[:].rearrange("p (e c) -> p e c", e=NCORES),
                    ACT.Sqrt,
                    scale=4096.0,
                )

            # ---- merged pass: degree + transpose-all; half-collectives ----
            for mt in range(MT):
                dcols = smallp.tile([P, NACH], F32, tag="dcols")
                for c in range(NACH):
                    ach = astreamp.tile([P, ACH], F32, tag="ach")
                    nc.sync.dma_start(
                        ach[:], a_d.ap()[mt * P : (mt + 1) * P, c * ACH : (c + 1) * ACH]
                    )
                    ach8 = astreamp.tile([P, ACH], F8, tag="ach8", bufs=2)
                    nc.scalar.activation(
                        ach8[:], ach[:], ACT.Copy, accum_out=dcols[:, c : c + 1]
                    )
                    for g in range(GPC):
                        gk = c * GPC + g  # k-group index 0..15
                        trp = psA.tile([P, 4 * P], F32, tag="trp")
                        for q in range(4):
                            nc.tensor.matmul(
                                trp[:, q * P : (q + 1) * P],
                                ach8[:, (g * 4 + q) * P : (g * 4 + q + 1) * P],
                                identity_f8[:],
                            )
                        dst = at_res[
                            :, (mt * KC + gk * 4) * P : (mt * KC + gk * 4 + 4) * P
                        ]
                        nc.vector.tensor_copy(dst, trp[:])
                nc.vector.reduce_sum(degree_sb[:, mt : mt + 1], dcols[:], axis=AX)
                if mt == HMT - 1:
                    issue_degree_half(0)
            issue_degree_half(1)

            recip8 = degp.tile([P, MT], F32)
            nc.vector.reciprocal(recip8[:], degree_sb[:])
            # dsown64[p, mt] = 1 / (64 * sqrt(degree_own[mt*128 + p]))
            dsown64 = degp.tile([P, MT], F32)
            nc.scalar.activation(dsown64[:], recip8[:], ACT.Sqrt, scale=1.0 / 4096.0)

            # ---- f: stream + park as raw fp8 + scale to X'' + matmul, fully
            # interleaved per 2-chunk batch so every engine queue's order
            # matches data arrival. The first fch buffers take a fake WAW dep
            # on degree_sb so the f stream cannot start before the A stream
            # is done.
            f8raw = f8pool.tile([P, KC * D], F8)
            xp_sb = xpp.tile([P, KC * D], F8)  # chunk kc at [:, kc*D:(kc+1)*D]
            # sub-batch (h, e, s) = k-chunks e*8 + h*4 + s*2 + {0,1} = pair
            # pj = 4e + 2h + s
            f_blk = f_d.ap().rearrange(
                "(e h s c p) d -> h e s p c d", h=2, s=2, c=2, p=P
            )

            def mm_pair(y_ap, mt, pj, start, stop):
                """pj = global k-pair index (k-chunks 2*pj, 2*pj+1)."""
                base = (mt * KC + 2 * pj) * P
                at2 = at_res[:, base : base + 2 * P].rearrange(
                    "p (k m) -> p k m", k=2
                )
                xp2 = xp_sb[:, (2 * pj) * D : (2 * pj + 2) * D].rearrange(
                    "p (k n) -> p k n", k=2
                )
                nc.tensor.matmul(
                    y_ap, at2, xp2, start=start, stop=stop, perf_mode=DR
                )

            ys = [
                psY.tile([P, D], F32, tag="y", name=f"y{i}") for i in range(MTG)
            ]
            # m-tiles 6,7 accumulate in psA's banks (free once the phase-1
            # transposes finish); their half-0 matmuls bridge the second
            # collective's latency window
            ys2 = [
                psA.tile([P, D], F32, tag="trp", name=f"y{MTG + i}")
                for i in range(MT - MTG)
            ]

            def g2_half(h):
                for e in range(NCORES):
                    for s in range(2):
                        pj = 4 * e + 2 * h + s
                        for i, mt in enumerate(range(MTG, MT)):
                            mm_pair(
                                ys2[i][:], mt, pj, pj == 0, pj == KC // 2 - 1
                            )

            nfch = [0]

            def stream_half(h):
                consume_degree_half(h)
                for e in range(NCORES):
                    for s in range(2):
                        pj = 4 * e + 2 * h + s
                        fch = fstreamp.tile([P, 2 * D], F32, tag="fch")
                        if nfch[0] < 4:
                            nc.vector.tensor_copy(
                                fch[:, :1], degree_sb[:, MT - 1 : MT]
                            )
                        nfch[0] += 1
                        nc.sync.dma_start(
                            fch[:].rearrange("p (c d) -> p c d", c=2),
                            f_blk[h, e, s],
                        )
                        for c in range(2):
                            kc = 2 * pj + c
                            # park on DVE (never waits the collective, so the
                            # f stream keeps flowing through its window)
                            nc.vector.tensor_copy(
                                f8raw[:, kc * D : (kc + 1) * D],
                                fch[:, c * D : (c + 1) * D],
                            )
                        for c in range(2):
                            kc = 2 * pj + c
                            # X'' on ScalarE: out = f8 * ds64 (scale is a
                            # per-partition AP)
                            nc.scalar.activation(
                                xp_sb[:, kc * D : (kc + 1) * D],
                                f8raw[:, kc * D : (kc + 1) * D],
                                ACT.Copy,
                                scale=ds64_sb[:, kc : kc + 1],
                            )
                        for mi in range(MTG):
                            mm_pair(ys[mi][:], mi, pj, pj == 0, pj == KC // 2 - 1)

            stream_half(0)
            g2_half(0)
            stream_half(1)

            # g2's half-1 matmuls get interleaved into the epilogue's PE gaps
            g2q = [
                (i, mt2, 4 * e + 2 + s)
                for e in range(NCORES)
                for s in range(2)
                for i, mt2 in enumerate(range(MTG, MT))
            ]
            gpos = [0]

            def pop_g2(n):
                for _ in range(n):
                    if gpos[0] < len(g2q):
                        i, mt2, pj = g2q[gpos[0]]
                        gpos[0] += 1
                        mm_pair(ys2[i][:], mt2, pj, False, pj == KC // 2 - 1)

            def epi_part1(mt, y_ap):
                res = epip.tile([P, D], F32, tag="res")
                nc.sync.dma_start(res[:], fres_d.ap()[mt * P : (mt + 1) * P, :])
                mf = mfpool.tile([P, D], BF16, tag="mf", name=f"mf{mt}")
                nc.vector.scalar_tensor_tensor(
                    mf[:],
                    y_ap,
                    dsown64[:, mt : mt + 1],
                    res[:],
                    op0=ALU.mult,
                    op1=ALU.add,
                )
                return mf

            def epi_part2(mt, mf):
                """out = relu(mf @ W + b); o + mfT rotate through psY slots."""
                o_ps = psY.tile([P, D], F32, tag="y", name=f"o{mt}")
                for wc in range(4):
                    mfT_ps = psY.tile([P, P], F32, tag="y", name=f"mfT{mt}_{wc}")
                    nc.tensor.matmul(
                        mfT_ps[:], mf[:, wc * P : (wc + 1) * P], identity_bf[:]
                    )
                    pop_g2(2)
                    mfT_sb = mftp.tile([P, P], BF16, tag="mfT")
                    nc.vector.tensor_copy(mfT_sb[:], mfT_ps[:])
                    nc.tensor.matmul(
                        o_ps[:],
                        mfT_sb[:],
                        w_sb[:, wc * D : (wc + 1) * D],
                        start=(wc == 0),
                        stop=False,
                    )
                nc.tensor.matmul(
                    o_ps[:], ones_row[:], b_sb[:], start=False, stop=True
                )
                osb = epip.tile([P, D], F32, tag="osb")
                nc.scalar.activation(osb[:], o_ps[:], ACT.Relu)
                nc.sync.dma_start(out_d.ap()[mt * P : (mt + 1) * P, :], osb[:])

            mfs = [epi_part1(mt, ys[mt][:]) for mt in range(MTG)]
            for mt in range(MTG):
                epi_part2(mt, mfs[mt])
            pop_g2(len(g2q))
            for i, mt in enumerate(range(MTG, MT)):
                mf = epi_part1(mt, ys2[i][:])
                epi_part2(mt, mf)

    nc.compile()
    return nc


def _get_nc():
    if "nc" not in _NC_CACHE:
        _NC_CACHE["nc"] = _build()
    return _NC_CACHE["nc"]


def run(inputs, trace=False, trace_kwargs=None):
    """Run the SPMD kernel; returns (full_output, BassKernelResults)."""
    a = np.ascontiguousarray(np.asarray(inputs["adjacency_matrix"], dtype=np.float32))
    f = np.ascontiguousarray(np.asarray(inputs["feature"], dtype=np.float32))
    w = np.ascontiguousarray(np.asarray(inputs["W"], dtype=np.float32))
    b = np.ascontiguousarray(np.asarray(inputs["b"], dtype=np.float32)).reshape(1, D)

    nc = _get_nc()
    in_maps = []
    for d in range(NCORES):
        rows = slice(d * R, (d + 1) * R)
        in_maps.append({"a": a[rows], "f": f, "fres": f[rows], "w": w, "bias": b})
    res = bass_utils.run_bass_kernel_spmd(
        nc,
        in_maps,
        core_ids=list(range(NCORES)),
        trace=trace,
        **(trace_kwargs or {}),
    )
    out = np.concatenate([r["out"] for r in res.results], axis=0)
    return out, res


def kernel(**inputs):
    out, _ = run(inputs, trace=False)
    return out
